# revision 20
# baseline (speedup 1.0000x reference)
"""Trainium2 Bass kernel for a 2-layer "BiGRU" (batch-flipped, per reference).

Structure exploited:
  * The reference's "backward" direction flips the BATCH dim, not time. In
    flipped coordinates (track hb_hat[b] := hb[B-1-b]) every GRU cell
    consumes the UNFLIPPED input stream; flips appear only when building
    layer-1's input concat and in the final output (host side).
  * Batch 64 is sharded over 8 cores in flip-closed groups of 8, so the
    flip is a local batch reversal and cores are fully independent.
  * All four GRU cells live in one 104-partition band layout
    (f0@0:8, b0@32:40, f1@64:72, b1@96:104). Layer 1 runs LAG steps behind
    layer 0 in the same iteration. The four cells' recurrent weight streams
    run in four PE column strips concurrently (interleaved quad emission);
    the elementwise gate math covers all four cells per op. The hidden dim
    is half-split (256-col ops) so each step's first half telescopes with
    the previous step's tail.
  * Input-side matmuls (x@wihT + biases) are bulk-precomputed at full PE
    width: layer-0's in a prepass; layer-1's in CH-step chunks as layer 0
    completes them. gi values round-trip DRAM and are DMA-gathered into a
    per-window "ring" tile in band layout; a single K=104 identity matmul
    injects them into the PSUM accumulation.

Self-contained: hardcodes all shapes from the problem spec.
"""

import numpy as np

from concourse import bacc, tile
from concourse.bass import mybir

SEQ, BATCH, IN, HID = 512, 64, 512, 512
G3 = 3 * HID  # 1536
BC = 8        # local batch per core
NCORES = 8
CH = 16       # wavefront chunk (steps) for layer-1 input bulk matmuls
LAG = 24      # layer-1 lag behind layer-0 (> CH + bulk spread, multiple of W4)
W4 = 2        # gi DMA window (steps)
FP32 = mybir.dt.float32
BF16 = mybir.dt.bfloat16

# cell name, partition band base, input K-chunks of 128
CELLS = [("f0", 0, 4), ("b0", 32, 4), ("f1", 64, 8), ("b1", 96, 8)]


def _blob_layout():
    """Free-dim offsets (in bf16 elements) inside the single load blob."""
    off = {}
    cur = 0
    for cname, _, kx in CELLS:
        for k in range(kx):
            off[f"w_{cname}{k}"] = cur
            cur += G3
        for k in range(4):
            off[f"u_{cname}{k}"] = cur
            cur += G3
    off["bias"] = cur      # rows 0:4 = cells; per cell: [bulk_bias 1536 | bhn 512]
    cur += 2048
    off["ohrow"] = cur     # rows 0:4; cell c: cols c*128..+128 = 1.0 in row c
    cur += 4 * 128
    off["i104"] = cur      # identity at rows/cols 0:104
    cur += 104
    off["sel4"] = cur      # (4, 104): row c one-hot over cell c's band cols
    cur += 104
    return off, cur


def build_core_program(S, repeats=1):
    assert S % CH == 0 and LAG % W4 == 0 and CH % W4 == 0
    nc = bacc.Bacc(None, target_bir_lowering=False)

    off, totw = _blob_layout()
    blob_d = nc.declare_dram_parameter("blob", [128, totw // 2], FP32, isOutput=False)
    xTp_d = nc.declare_dram_parameter("xTp", [128, 4, S * BC // 2], FP32, isOutput=False)
    out_d = nc.declare_dram_parameter("out", [S, 40, HID], BF16, isOutput=True)

    with tile.TileContext(nc) as tc:
        for _ in range(repeats):
            build_body(nc, tc, S, blob_d, xTp_d, out_d, off, totw)
    nc.compile()
    return nc


def build_body(nc, tc, S, blob_d, xTp_d, out_d, off, totw):
    import contextlib

    ACT = mybir.ActivationFunctionType
    OP = mybir.AluOpType
    NCHUNK = S // CH
    P104 = slice(0, 104)

    ctx = contextlib.ExitStack()
    with ctx:
        const = ctx.enter_context(tc.tile_pool(name="const", bufs=1))
        ghp = ctx.enter_context(tc.tile_pool(name="ghp", bufs=1, space="PSUM"))
        ptrp = ctx.enter_context(tc.tile_pool(name="ptrp", bufs=2, space="PSUM"))
        scr = ctx.enter_context(tc.tile_pool(name="scr", bufs=1, space="PSUM"))
        warmp = ctx.enter_context(tc.tile_pool(name="warmp", bufs=1, space="PSUM"))
        dram = ctx.enter_context(tc.tile_pool(name="dram", bufs=1, space="DRAM"))
        xr_pool = ctx.enter_context(tc.tile_pool(name="xr", bufs=2))
        ev_pool = ctx.enter_context(tc.tile_pool(name="ev", bufs=2))
        ring_pool = ctx.enter_context(tc.tile_pool(name="ring", bufs=3))
        buf_pool = ctx.enter_context(tc.tile_pool(name="buf", bufs=3))
        hT_pool = ctx.enter_context(tc.tile_pool(name="hT", bufs=3))
        g_pool = ctx.enter_context(tc.tile_pool(name="g", bufs=2))
        h2_pool = ctx.enter_context(tc.tile_pool(name="h2", bufs=3))

        # ---- load blob (single DMA), bf16 views via bitcast ----
        blob = const.tile([128, totw // 2], FP32, tag="blob")
        nc.gpsimd.dma_start(out=blob[:], in_=blob_d[:])
        b16 = blob[:].bitcast(BF16)

        W, U, BULKB, OHR = {}, {}, {}, {}
        ob, oh = off["bias"], off["ohrow"]
        for ci, (cname, base, kx) in enumerate(CELLS):
            W[cname] = [b16[:, off[f"w_{cname}{k}"]:off[f"w_{cname}{k}"] + G3]
                        for k in range(kx)]
            U[cname] = [b16[:, off[f"u_{cname}{k}"]:off[f"u_{cname}{k}"] + G3]
                        for k in range(4)]
            BULKB[cname] = b16[0:4, ob:ob + 1536]      # row ci is live
            OHR[cname] = b16[0:4, oh + ci * 128:oh + (ci + 1) * 128]
        BHNROWS = b16[0:4, ob + 1536:ob + 2048]
        I104 = b16[0:104, off["i104"]:off["i104"] + 104]
        SEL4 = b16[0:4, off["sel4"]:off["sel4"] + 104]

        zeroH = const.tile([128, 512], BF16, tag="zeroH")
        nc.any.memset(zeroH[:], 0.0)
        warm = warmp.tile([128, 512], FP32, tag="warm")

        def keep_warm(n_dummy, rhs):
            # tiny matmuls into a scratch bank, with a data dependency on a
            # mid-chain tile: they self-schedule into the PE's wait-for-gates
            # gap so the HAM activity monitor never sees an idle window and
            # the PE clock stays at 2.4 GHz.
            dlhs = b16[0:8, off["i104"]:off["i104"] + 8]
            for _ in range(n_dummy):
                nc.tensor.matmul(out=warm[0:8, :], lhsT=dlhs,
                                 rhs=rhs, start=True, stop=True)

        # ---- internal DRAM for bulk gi results ----
        gi0_dram = {c: dram.tile([S * BC, G3], BF16, tag=f"gi0_{c}", name=f"gi0_{c}",
                                 uniquify=True)
                    for c in ("f0", "b0")}
        gi1_dram = {c: [dram.tile([CH * BC, G3], BF16, tag=f"gi1_{c}", bufs=4,
                                  name=f"gi1_{c}_{cc}")
                        for cc in range(NCHUNK)]
                    for c in ("f1", "b1")}

        def bulk_group(cell, lhs_chunks, n):
            """PSUM matmuls for one 512-col slice of gi = x @ wihT + bias.
            Returns the PSUM tile; evacuation is the caller's job (deferred
            to the end of the iteration so it stays off the gate chain)."""
            ps = scr.tile([128, 512], FP32, tag="scr", bufs=2)
            for k, lhs in enumerate(lhs_chunks):
                nc.tensor.matmul(out=ps[:], lhsT=lhs,
                                 rhs=W[cell][k][:, n * 512:(n + 1) * 512],
                                 start=(k == 0), stop=False)
            nc.tensor.matmul(out=ps[:], lhsT=OHR[cell],
                             rhs=BULKB[cell][:, n * 512:(n + 1) * 512],
                             start=False, stop=True)
            return ps

        def evac_group(ps, out_rows_ap, n):
            ev = ev_pool.tile([128, 512], BF16, tag="ev")
            nc.scalar.activation(ev[:], ps[:], ACT.Copy)
            nc.gpsimd.dma_start(out=out_rows_ap[:, n * 512:(n + 1) * 512],
                                in_=ev[:])

        def load_xchunk(c):
            xrt = xr_pool.tile([128, 4, 64], FP32, tag="xr", name=f"xr{c}")
            nc.gpsimd.dma_start(out=xrt[:], in_=xTp_d[:, :, c * 64:(c + 1) * 64])
            return xrt[:].bitcast(BF16)   # (128, 4, 128)

        # ---- mini-prepass: gi0 for chunk 0 only (the rest interleaves) ----
        x16 = load_xchunk(0)
        for cell in ("f0", "b0"):
            for n in range(3):
                ps = bulk_group(cell, [x16[:, k, :] for k in range(4)], n)
                evac_group(ps, gi0_dram[cell][0:128, :], n)

        # ---- wavefront loop: L0 at step i, L1 at step i-LAG ----
        bufA, bufC = {}, {}   # chunk -> (128, 4, CH*BC) tiles (L0 h, hidden-major)

        def dma_ring(iw):
            """Prefetch one W4-step window of gi slices for both layers."""
            t0w, t1w = iw, iw - LAG
            r = ring_pool.tile([128, W4 * G3], BF16, tag="ring",
                               name=f"ring{iw}")
            if iw < 3 * W4:
                # first touch of each ring slot: zero so the injects never
                # read garbage rows
                nc.any.memset(r[:], 0.0)
            rv = r.rearrange("p (s g) -> p s g", s=W4)
            if 0 <= t0w < S:
                for cell, base in (("f0", 0), ("b0", 32)):
                    src = gi0_dram[cell][:].rearrange(
                        "(s b) g -> b s g", b=BC)[:, t0w:t0w + W4, :]
                    nc.sync.dma_start(out=rv[base:base + BC], in_=src)
            if 0 <= t1w < S:
                for cell, base in (("f1", 64), ("b1", 96)):
                    src = gi1_dram[cell][t1w // CH][:].rearrange(
                        "(s b) g -> b s g", b=BC)[:, t1w % CH:t1w % CH + W4, :]
                    nc.sync.dma_start(out=rv[base:base + BC], in_=src)
            elif t1w < 0 and iw >= 3 * W4:
                # keep L1's path exactly zero until its t=0 arrives
                nc.any.memset(r[64:104, :], 0.0)
            return r

        ring_next = dma_ring(0)
        ring = None
        hT_prev = None
        h2_prev = None
        x16_next = None
        for i in range(S + LAG):
            t0, t1 = i, i - LAG         # layer-0 / layer-1 step indices
            cc0 = t0 // CH

            if i % W4 == 0:
                ring = ring_next
                ring_next = dma_ring(i + W4) if i + W4 < S + LAG else None
            wi = i % W4

            l0 = 0 <= t0 < S
            l1 = 0 <= t1 < S
            active = [c for c, l in zip(CELLS, (l0, l0, l1, l1)) if l]

            if l0 and t0 % CH == 0:
                bufA[cc0] = buf_pool.tile([128, 4, CH * BC], BF16, tag="bufA",
                                          name=f"bufA{cc0}")
                bufC[cc0] = buf_pool.tile([128, 4, CH * BC], BF16, tag="bufC",
                                          name=f"bufC{cc0}")

            def hch(base, k):
                if hT_prev is None:
                    return zeroH[:, 0:BC]
                return hT_prev[:, k, base:base + BC]

            # ---------- gh = gi + bias + h @ whhT  (PSUM, all four bands) ----
            gh = ghp.tile([104, 1536], FP32, tag="gh", bufs=1, name=f"gh{i}")
            rb = wi * G3
            nc.tensor.matmul(out=gh[P104, 0:512], lhsT=I104,
                             rhs=ring[0:104, rb:rb + 512], start=True,
                             stop=False)
            nc.tensor.matmul(out=gh[P104, 512:1024], lhsT=I104,
                             rhs=ring[0:104, rb + 512:rb + 1024], start=True,
                             stop=False)
            if l1:
                nc.tensor.matmul(out=gh[P104, 1024:1536], lhsT=SEL4,
                                 rhs=BHNROWS, start=True, stop=False)
            else:
                # L1 not yet live: its n-slice must be exactly zero so the
                # L1 hidden state stays zero until t1 = 0 (ring is zeroed).
                nc.tensor.matmul(out=gh[0:40, 1024:1536], lhsT=SEL4[:, 0:40],
                                 rhs=BHNROWS, start=True, stop=False)
                nc.tensor.matmul(out=gh[64:104, 1024:1536],
                                 lhsT=b16[64:104, off["i104"] + 64:
                                          off["i104"] + 104],
                                 rhs=ring[64:104, rb + 1024:rb + 1536],
                                 start=True, stop=False)
            # slice blocks in chain order r, z, n; k-major quads inside each
            # block so the four cells' streams run in four col strips.
            for n_lo in (0, 1024, 512):
                for k in range(4):
                    for cname, base, _ in active:
                        nc.tensor.matmul(
                            out=gh[base:base + BC, n_lo:n_lo + 512],
                            lhsT=hch(base, k),
                            rhs=U[cname][k][:, n_lo:n_lo + 512],
                            start=False, stop=(k == 3),
                            tile_position=(0, base))

            # ---------- bulk gi matmuls: PE gap fillers between this step's
            # MM block and its transposes. gi1 for the L1 wavefront on
            # t0%CH in [0,6); gi0 for the next x-chunk on t0%CH in [6,12).
            # Evacuations are deferred to the end of the iteration.
            evacs = []
            bc = (t0 - CH) // CH          # chunk fully copied CH iters ago
            ph = (t0 - CH) % CH
            if 0 <= bc < NCHUNK and ph < 6:
                lhs = [bufA[bc][:, k, :] for k in range(4)] + \
                      [bufC[bc][:, k, :] for k in range(4)]
                cell = ("f1", "b1")[ph // 3]
                n = ph % 3
                ps = bulk_group(cell, lhs, n)
                evacs.append((ps, gi1_dram[cell][bc], n))
            c0 = t0 // CH + 1             # gi0 for the next chunk
            ph0 = t0 % CH
            if l0 and c0 < NCHUNK:
                if ph0 == 5:
                    x16_next = load_xchunk(c0)
                elif 6 <= ph0 < 12:
                    j = ph0 - 6
                    cell = ("f0", "b0")[j // 3]
                    n = j % 3
                    ps = bulk_group(cell, [x16_next[:, k, :] for k in range(4)], n)
                    evacs.append((ps, gi0_dram[cell][c0 * 128:(c0 + 1) * 128, :], n))

            # ---------- gates: all four cells per op, full 512-col ops ----
            h_prev = h2_prev[:] if h2_prev is not None else zeroH[0:104, :]
            rz = g_pool.tile([104, 1024], BF16, tag="rz")   # r 0:512, z 512:1024
            nc.scalar.activation(rz[:, 0:512], gh[P104, 0:512], ACT.Sigmoid)
            nc.scalar.activation(rz[:, 512:1024], gh[P104, 512:1024], ACT.Sigmoid)

            zb = g_pool.tile([104, 512], BF16, tag="zb")
            m2 = g_pool.tile([104, 512], BF16, tag="m2")
            u = g_pool.tile([104, 512], BF16, tag="u")
            v = g_pool.tile([104, 512], BF16, tag="v")
            nt = g_pool.tile([104, 512], BF16, tag="nt")
            nb = g_pool.tile([104, 512], BF16, tag="nb")
            h2 = h2_pool.tile([104, 512], BF16, tag="h2", name=f"h2_{i}")

            # z-branch on the (otherwise idle) GPSIMD engine so the DVE
            # queue stays short on the critical path.
            nc.vector.tensor_tensor(out=u[:], in0=rz[:, 0:512],
                                    in1=gh[P104, 1024:1536], op=OP.mult)
            nc.vector.tensor_tensor(out=v[:], in0=u[:],
                                    in1=ring[0:104, rb + 1024:rb + 1536],
                                    op=OP.add)
            nc.vector.tensor_scalar(out=zb[:], in0=rz[:, 512:1024],
                                    scalar1=-1.0, scalar2=1.0,
                                    op0=OP.mult, op1=OP.add)
            nc.vector.tensor_tensor(out=m2[:], in0=rz[:, 512:1024],
                                    in1=h_prev[0:104, :], op=OP.mult)
            nc.scalar.activation(nt[:], v[:], ACT.Tanh)
            nc.vector.tensor_tensor(out=nb[:], in0=nt[:], in1=zb[:],
                                    op=OP.mult)
            nc.vector.tensor_tensor(out=h2[0:104, 0:256], in0=nb[:, 0:256],
                                    in1=m2[:, 0:256], op=OP.add)
            nc.vector.tensor_tensor(out=h2[0:104, 256:512], in0=nb[:, 256:512],
                                    in1=m2[:, 256:512], op=OP.add)

            # PE gap fillers pinned to the gate chain's phase
            keep_warm(2 if evacs else 6, rz[0:8, 0:512])
            keep_warm(2 if evacs else 4, nt[0:8, 0:512])

            # ---------- transpose h2 -> hidden-major for next step ----
            ptrA = ptrp.tile([128, 2, 104], BF16, tag="ptrA", bufs=1,
                             name=f"ptrA{i}")
            ptrB = ptrp.tile([128, 2, 104], BF16, tag="ptrB", bufs=1,
                             name=f"ptrB{i}")
            for k in range(4):
                nc.tensor.transpose(out=(ptrA if k < 2 else ptrB)[:, k % 2, 0:104],
                                    in_=h2[0:104, k * 128:(k + 1) * 128],
                                    identity=I104)
            hT = hT_pool.tile([128, 4, 104], BF16, tag="hT", name=f"hT{i}")
            nc.vector.tensor_copy(out=hT[:, 0:2, :], in_=ptrA[:])
            nc.vector.tensor_copy(out=hT[:, 2:4, :], in_=ptrB[:])
            hT_prev = hT
            h2_prev = h2

            if l1:
                nc.gpsimd.dma_start(out=out_d[t1], in_=h2[64:104, :])
            if l0:
                sl = (t0 % CH) * BC
                nc.gpsimd.tensor_copy(out=bufA[cc0][:, :, sl:sl + BC],
                                      in_=hT[:, :, 0:BC])
                nc.gpsimd.tensor_copy(out=bufC[cc0][:, :, sl:sl + BC],
                                      in_=hT[:, :, 39:31:-1])

            for ps, dst, n in evacs:
                evac_group(ps, dst, n)


# ---------------------------------------------------------------------------
# host side
# ---------------------------------------------------------------------------

_CACHE = {}


def _groups():
    return [list(range(4 * d, 4 * d + 4)) + [63 - (4 * d + 3), 63 - (4 * d + 2),
            63 - (4 * d + 1), 63 - 4 * d] for d in range(NCORES)]


def _bf16_u16(a):
    a = np.ascontiguousarray(a, np.float32)
    u = a.view(np.uint32)
    return ((u + 0x7FFF + ((u >> 16) & 1)) >> 16).astype(np.uint16)


def _pack_words(u16):
    ev = u16[..., 0::2].astype(np.uint32)
    od = u16[..., 1::2].astype(np.uint32)
    return (ev | (od << 16)).view(np.float32)


def _blob_host(inputs):
    off, totw = _blob_layout()
    blob = np.zeros((128, totw), np.uint16)
    one = _bf16_u16(np.ones(1, np.float32))[0]
    for ci, (cname, base, kx) in enumerate(CELLS):
        wih = np.asarray(inputs[f"wih_{cname}"], np.float32)   # (1536, in)
        whh = np.asarray(inputs[f"whh_{cname}"], np.float32)   # (1536, 512)
        bih = np.asarray(inputs[f"bih_{cname}"], np.float32)
        bhh = np.asarray(inputs[f"bhh_{cname}"], np.float32)
        wt = _bf16_u16(wih.T.reshape(kx, 128, G3))
        ut = _bf16_u16(whh.T.reshape(4, 128, G3))
        for k in range(kx):
            o = off[f"w_{cname}{k}"]
            blob[:, o:o + G3] = wt[k]
        for k in range(4):
            o = off[f"u_{cname}{k}"]
            blob[:, o:o + G3] = ut[k]
        bulkb = np.concatenate([(bih + bhh)[:1024], bih[1024:]])
        blob[ci, off["bias"]:off["bias"] + 1536] = _bf16_u16(bulkb)
        blob[ci, off["bias"] + 1536:off["bias"] + 2048] = _bf16_u16(bhh[1024:])
        blob[ci, off["ohrow"] + ci * 128:off["ohrow"] + (ci + 1) * 128] = \
            _bf16_u16(np.ones(128, np.float32))
    for j in range(104):
        blob[j, off["i104"] + j] = one
    for ci, (cname, base, kx) in enumerate(CELLS):
        for j in range(BC):
            blob[ci, off["sel4"] + base + j] = one
    return _pack_words(blob)


def _in_maps(inputs):
    S = inputs["x"].shape[0]
    x = np.asarray(inputs["x"], np.float32)
    groups = _groups()
    blob = _blob_host(inputs)
    in_maps = []
    for d in range(NCORES):
        xl = x[:, groups[d], :]                       # (S, 8, 512)
        # xTp layout: (128 part, 4 k, S*BC) -> words
        xT = _bf16_u16(xl.transpose(2, 0, 1).reshape(4, 128, S * BC))
        xT = np.ascontiguousarray(xT.transpose(1, 0, 2))   # (128, 4, S*BC)
        in_maps.append({"blob": blob, "xTp": _pack_words(xT)})
    return in_maps


def _assemble(outs, S):
    groups = _groups()
    out = np.zeros((S, BATCH, 2 * HID), np.float32)
    for d in range(NCORES):
        raw = np.asarray(outs[d]["out"], np.float32)  # (S, 40, 512)
        G = groups[d]
        for b in range(BC):
            out[:, G[b], 0:HID] = raw[:, b, :]
            out[:, G[b], HID:] = raw[:, 32 + 7 - b, :]
    return out


class _Runner:
    """Caches the traced+compiled SPMD executable so repeat calls skip the
    (expensive) jax retrace and BIR re-serialization."""

    def __init__(self, S):
        import jax
        from jax.sharding import Mesh, PartitionSpec
        from jax.experimental.shard_map import shard_map
        from concourse import bass2jax
        from concourse.bass2jax import _bass_exec_p, partition_id_tensor

        bass2jax.install_neuronx_cc_hook()
        self.S = S
        nc = build_core_program(S)
        self.nc = nc
        partition_name = nc.partition_id_tensor.name if nc.partition_id_tensor else None
        in_names, out_names, out_avals, zero_outs = [], [], [], []
        for alloc in nc.m.functions[0].allocations:
            if not isinstance(alloc, mybir.MemoryLocationSet):
                continue
            name = alloc.memorylocations[0].name
            if alloc.kind == "ExternalInput":
                if name != partition_name:
                    in_names.append(name)
            elif alloc.kind == "ExternalOutput":
                shape = tuple(alloc.tensor_shape)
                dtype = mybir.dt.np(alloc.dtype)
                out_names.append(name)
                out_avals.append(jax.core.ShapedArray(shape, dtype))
                zero_outs.append(np.zeros(shape, dtype))
        n_params = len(in_names)
        self.in_names = list(in_names)
        self.out_names = out_names
        self.out_shapes = [tuple(a.shape) for a in out_avals]
        self.zero_outs = zero_outs
        all_in = in_names + out_names + ([partition_name] if partition_name else [])

        def _body(*args):
            operands = list(args)
            if partition_name is not None:
                operands.append(partition_id_tensor())
            return tuple(_bass_exec_p.bind(
                *operands,
                out_avals=tuple(out_avals),
                in_names=tuple(all_in),
                out_names=tuple(out_names),
                lowering_input_output_aliases=(),
                sim_require_finite=True,
                sim_require_nnan=True,
                nc=nc,
            ))

        devices = jax.devices()[:NCORES]
        mesh = Mesh(np.asarray(devices), ("core",))
        in_specs = (PartitionSpec("core"),) * (n_params + len(out_names))
        out_specs = (PartitionSpec("core"),) * len(out_names)
        self.fn = jax.jit(
            shard_map(_body, mesh=mesh, in_specs=in_specs,
                      out_specs=out_specs, check_rep=False),
            keep_unused=True)
        self.jax = jax

    def run(self, in_maps):
        concat_in = [
            np.concatenate([np.asarray(m[nm]) for m in in_maps], axis=0)
            for nm in self.in_names]
        concat_zero = [np.zeros((NCORES * z.shape[0], *z.shape[1:]), z.dtype)
                       for z in self.zero_outs]
        outs = self.fn(*concat_in, *concat_zero)
        return [
            {nm: np.asarray(outs[i]).reshape(NCORES, *self.out_shapes[i])[c]
             for i, nm in enumerate(self.out_names)}
            for c in range(NCORES)]

    def run_timed(self, in_maps, iters=5):
        """Stage inputs (and the pre-zeroed output buffers — every output
        element is written, so reuse is safe) on device; time executions."""
        import time
        concat_in = [
            self.jax.device_put(np.concatenate(
                [np.asarray(m[nm]) for m in in_maps], axis=0))
            for nm in self.in_names]
        concat_zero = [
            self.jax.device_put(
                np.zeros((NCORES * z.shape[0], *z.shape[1:]), z.dtype))
            for z in self.zero_outs]
        o = self.fn(*concat_in, *concat_zero)
        self.jax.block_until_ready(o)
        best = float("inf")
        for _ in range(iters):
            t0 = time.perf_counter()
            o = self.fn(*concat_in, *concat_zero)
            self.jax.block_until_ready(o)
            best = min(best, time.perf_counter() - t0)
        return best


def kernel(**inputs):
    S = inputs["x"].shape[0]
    if S not in _CACHE:
        _CACHE[S] = _Runner(S)
    runner = _CACHE[S]
    outs = runner.run(_in_maps(inputs))
    return _assemble(outs, S)


if __name__ == "__main__":
    rng = np.random.default_rng(0)
    S = 32
    inputs = {"x": rng.standard_normal((S, 64, 512), dtype=np.float32)}
    s = 1.0 / np.sqrt(HID)
    u = lambda *shp: rng.uniform(-s, s, shp).astype(np.float32)
    for c, idim in (("f0", 512), ("b0", 512), ("f1", 1024), ("b1", 1024)):
        inputs[f"wih_{c}"] = u(G3, idim)
        inputs[f"whh_{c}"] = u(G3, HID)
        inputs[f"bih_{c}"] = u(G3)
        inputs[f"bhh_{c}"] = u(G3)
    out = kernel(**inputs)
    print("kernel ran, out", out.shape, float(np.abs(out).mean()))


# revision 22
# speedup vs baseline: 1.1154x; 1.1154x over previous
"""Trainium2 Bass kernel for a 2-layer "BiGRU" (batch-flipped, per reference).

Structure exploited:
  * The reference's "backward" direction flips the BATCH dim, not time. In
    flipped coordinates (track hb_hat[b] := hb[B-1-b]) every GRU cell
    consumes the UNFLIPPED input stream; flips appear only when building
    layer-1's input concat and in the final output (host side).
  * Batch 64 is sharded over 8 cores in flip-closed groups of 8, so the
    flip is a local batch reversal and cores are fully independent.
  * All four GRU cells live in one 104-partition band layout
    (f0@0:8, b0@32:40, f1@64:72, b1@96:104). Layer 1 runs LAG steps behind
    layer 0 in the same iteration. The four cells' recurrent weight streams
    run in four PE column strips concurrently (interleaved quad emission);
    the elementwise gate math covers all four cells per op. The hidden dim
    is half-split (256-col ops) so each step's first half telescopes with
    the previous step's tail.
  * Input-side matmuls (x@wihT + biases) are bulk-precomputed at full PE
    width: layer-0's in a prepass; layer-1's in CH-step chunks as layer 0
    completes them. gi values round-trip DRAM and are DMA-gathered into a
    per-window "ring" tile in band layout; a single K=104 identity matmul
    injects them into the PSUM accumulation.

Self-contained: hardcodes all shapes from the problem spec.
"""

import numpy as np

from concourse import bacc, tile
from concourse.bass import mybir

SEQ, BATCH, IN, HID = 512, 64, 512, 512
G3 = 3 * HID  # 1536
BC = 8        # local batch per core
NCORES = 8
CH = 16       # wavefront chunk (steps) for layer-1 input bulk matmuls
LAG = 24      # layer-1 lag behind layer-0 (> CH + bulk spread, multiple of W4)
W4 = 2        # gi DMA window (steps)
FP32 = mybir.dt.float32
BF16 = mybir.dt.bfloat16

# cell name, partition band base, input K-chunks of 128
CELLS = [("f0", 0, 4), ("b0", 32, 4), ("f1", 64, 8), ("b1", 96, 8)]


def _blob_layout():
    """Free-dim offsets (in bf16 elements) inside the single load blob."""
    off = {}
    cur = 0
    for cname, _, kx in CELLS:
        for k in range(kx):
            off[f"w_{cname}{k}"] = cur
            cur += G3
        for k in range(4):
            off[f"u_{cname}{k}"] = cur
            cur += G3
    off["bias"] = cur      # rows 0:4 = cells; per cell: [bulk_bias 1536 | bhn 512]
    cur += 2048
    off["ohrow"] = cur     # rows 0:4; cell c: cols c*128..+128 = 1.0 in row c
    cur += 4 * 128
    off["i104"] = cur      # identity at rows/cols 0:104
    cur += 104
    off["sel4"] = cur      # (4, 104): row c one-hot over cell c's band cols
    cur += 104
    return off, cur


def build_core_program(S, repeats=1):
    assert S % CH == 0 and LAG % W4 == 0 and CH % W4 == 0
    nc = bacc.Bacc(None, target_bir_lowering=False)

    off, totw = _blob_layout()
    blob_d = nc.declare_dram_parameter("blob", [128, totw // 2], FP32, isOutput=False)
    xTp_d = nc.declare_dram_parameter("xTp", [128, 4, S * BC // 2], FP32, isOutput=False)
    out_d = nc.declare_dram_parameter("out", [S, 40, HID], BF16, isOutput=True)

    with tile.TileContext(nc) as tc:
        for _ in range(repeats):
            build_body(nc, tc, S, blob_d, xTp_d, out_d, off, totw)
    nc.compile()
    return nc


def build_body(nc, tc, S, blob_d, xTp_d, out_d, off, totw):
    import contextlib

    ACT = mybir.ActivationFunctionType
    OP = mybir.AluOpType
    NCHUNK = S // CH
    P104 = slice(0, 104)

    ctx = contextlib.ExitStack()
    with ctx:
        const = ctx.enter_context(tc.tile_pool(name="const", bufs=1))
        ghp = ctx.enter_context(tc.tile_pool(name="ghp", bufs=1, space="PSUM"))
        ptrp = ctx.enter_context(tc.tile_pool(name="ptrp", bufs=2, space="PSUM"))
        scr = ctx.enter_context(tc.tile_pool(name="scr", bufs=1, space="PSUM"))
        warmp = ctx.enter_context(tc.tile_pool(name="warmp", bufs=1, space="PSUM"))
        dram = ctx.enter_context(tc.tile_pool(name="dram", bufs=1, space="DRAM"))
        xr_pool = ctx.enter_context(tc.tile_pool(name="xr", bufs=2))
        ev_pool = ctx.enter_context(tc.tile_pool(name="ev", bufs=2))
        ring_pool = ctx.enter_context(tc.tile_pool(name="ring", bufs=3))
        buf_pool = ctx.enter_context(tc.tile_pool(name="buf", bufs=3))
        hT_pool = ctx.enter_context(tc.tile_pool(name="hT", bufs=3))
        g_pool = ctx.enter_context(tc.tile_pool(name="g", bufs=2))
        h2_pool = ctx.enter_context(tc.tile_pool(name="h2", bufs=3))

        # ---- load blob (single DMA), bf16 views via bitcast ----
        blob = const.tile([128, totw // 2], FP32, tag="blob")
        nc.gpsimd.dma_start(out=blob[:], in_=blob_d[:])
        b16 = blob[:].bitcast(BF16)

        W, U, BULKB, OHR = {}, {}, {}, {}
        ob, oh = off["bias"], off["ohrow"]
        for ci, (cname, base, kx) in enumerate(CELLS):
            W[cname] = [b16[:, off[f"w_{cname}{k}"]:off[f"w_{cname}{k}"] + G3]
                        for k in range(kx)]
            U[cname] = [b16[:, off[f"u_{cname}{k}"]:off[f"u_{cname}{k}"] + G3]
                        for k in range(4)]
            BULKB[cname] = b16[0:4, ob:ob + 1536]      # row ci is live
            OHR[cname] = b16[0:4, oh + ci * 128:oh + (ci + 1) * 128]
        BHNROWS = b16[0:4, ob + 1536:ob + 2048]
        I104 = b16[0:104, off["i104"]:off["i104"] + 104]
        SEL4 = b16[0:4, off["sel4"]:off["sel4"] + 104]

        zeroH = const.tile([128, 512], BF16, tag="zeroH")
        nc.any.memset(zeroH[:], 0.0)
        warm = warmp.tile([128, 512], FP32, tag="warm")

        def keep_warm(n_dummy, rhs):
            # tiny matmuls into a scratch bank, with a data dependency on a
            # mid-chain tile: they self-schedule into the PE's wait-for-gates
            # gap so the HAM activity monitor never sees an idle window and
            # the PE clock stays at 2.4 GHz.
            dlhs = b16[0:8, off["i104"]:off["i104"] + 8]
            for _ in range(n_dummy):
                nc.tensor.matmul(out=warm[0:8, :], lhsT=dlhs,
                                 rhs=rhs, start=True, stop=True)

        # ---- internal DRAM for bulk gi results ----
        gi0_dram = {c: dram.tile([S * BC, G3], BF16, tag=f"gi0_{c}", name=f"gi0_{c}",
                                 uniquify=True)
                    for c in ("f0", "b0")}
        gi1_dram = {c: [dram.tile([CH * BC, G3], BF16, tag=f"gi1_{c}", bufs=4,
                                  name=f"gi1_{c}_{cc}")
                        for cc in range(NCHUNK)]
                    for c in ("f1", "b1")}

        def bulk_group(cell, lhs_chunks, n):
            """PSUM matmuls for one 512-col slice of gi = x @ wihT + bias.
            Returns the PSUM tile; evacuation is the caller's job (deferred
            to the end of the iteration so it stays off the gate chain)."""
            ps = scr.tile([128, 512], FP32, tag="scr", bufs=2)
            for k, lhs in enumerate(lhs_chunks):
                nc.tensor.matmul(out=ps[:], lhsT=lhs,
                                 rhs=W[cell][k][:, n * 512:(n + 1) * 512],
                                 start=(k == 0), stop=False)
            nc.tensor.matmul(out=ps[:], lhsT=OHR[cell],
                             rhs=BULKB[cell][:, n * 512:(n + 1) * 512],
                             start=False, stop=True)
            return ps

        def evac_group(ps, out_rows_ap, n):
            ev = ev_pool.tile([128, 512], BF16, tag="ev")
            nc.scalar.activation(ev[:], ps[:], ACT.Copy)
            nc.gpsimd.dma_start(out=out_rows_ap[:, n * 512:(n + 1) * 512],
                                in_=ev[:])

        def load_xchunk(c):
            xrt = xr_pool.tile([128, 4, 64], FP32, tag="xr", name=f"xr{c}")
            nc.gpsimd.dma_start(out=xrt[:], in_=xTp_d[:, :, c * 64:(c + 1) * 64])
            return xrt[:].bitcast(BF16)   # (128, 4, 128)

        # ---- mini-prepass: gi0 for chunk 0 only (the rest interleaves) ----
        x16 = load_xchunk(0)
        for cell in ("f0", "b0"):
            for n in range(3):
                ps = bulk_group(cell, [x16[:, k, :] for k in range(4)], n)
                evac_group(ps, gi0_dram[cell][0:128, :], n)

        # ---- wavefront loop: L0 at step i, L1 at step i-LAG ----
        bufA, bufC = {}, {}   # chunk -> (128, 4, CH*BC) tiles (L0 h, hidden-major)

        def dma_ring(iw):
            """Prefetch one W4-step window of gi slices for both layers."""
            t0w, t1w = iw, iw - LAG
            r = ring_pool.tile([128, W4 * G3], BF16, tag="ring",
                               name=f"ring{iw}")
            if iw < 3 * W4:
                # first touch of each ring slot: zero so the injects never
                # read garbage rows
                nc.any.memset(r[:], 0.0)
            rv = r.rearrange("p (s g) -> p s g", s=W4)
            if 0 <= t0w < S:
                for cell, base in (("f0", 0), ("b0", 32)):
                    src = gi0_dram[cell][:].rearrange(
                        "(s b) g -> b s g", b=BC)[:, t0w:t0w + W4, :]
                    nc.sync.dma_start(out=rv[base:base + BC], in_=src)
            if 0 <= t1w < S:
                for cell, base in (("f1", 64), ("b1", 96)):
                    src = gi1_dram[cell][t1w // CH][:].rearrange(
                        "(s b) g -> b s g", b=BC)[:, t1w % CH:t1w % CH + W4, :]
                    nc.sync.dma_start(out=rv[base:base + BC], in_=src)
            elif t1w < 0 and iw >= 3 * W4:
                # keep L1's path exactly zero until its t=0 arrives
                nc.any.memset(r[64:104, :], 0.0)
            return r

        ring_next = dma_ring(0)
        ring = None
        hT_prev = None
        h2_prev = None
        x16_next = None
        for i in range(S + LAG):
            t0, t1 = i, i - LAG         # layer-0 / layer-1 step indices
            cc0 = t0 // CH

            if i % W4 == 0:
                ring = ring_next
                ring_next = dma_ring(i + W4) if i + W4 < S + LAG else None
            wi = i % W4

            l0 = 0 <= t0 < S
            l1 = 0 <= t1 < S
            active = [c for c, l in zip(CELLS, (l0, l0, l1, l1)) if l]

            if l0 and t0 % CH == 0:
                bufA[cc0] = buf_pool.tile([128, 4, CH * BC], BF16, tag="bufA",
                                          name=f"bufA{cc0}")
                bufC[cc0] = buf_pool.tile([128, 4, CH * BC], BF16, tag="bufC",
                                          name=f"bufC{cc0}")

            def hch(base, k):
                if hT_prev is None:
                    return zeroH[:, 0:BC]
                return hT_prev[:, k, base:base + BC]

            # ---------- gh = gi + bias + h @ whhT  (PSUM, all four bands) ----
            gh = ghp.tile([104, 1536], FP32, tag="gh", bufs=1, name=f"gh{i}")
            rb = wi * G3
            nc.tensor.matmul(out=gh[P104, 0:512], lhsT=I104,
                             rhs=ring[0:104, rb:rb + 512], start=True,
                             stop=False)
            nc.tensor.matmul(out=gh[P104, 512:1024], lhsT=I104,
                             rhs=ring[0:104, rb + 512:rb + 1024], start=True,
                             stop=False)
            if l1:
                nc.tensor.matmul(out=gh[P104, 1024:1536], lhsT=SEL4,
                                 rhs=BHNROWS, start=True, stop=False)
            else:
                # L1 not yet live: its n-slice must be exactly zero so the
                # L1 hidden state stays zero until t1 = 0 (ring is zeroed).
                nc.tensor.matmul(out=gh[0:40, 1024:1536], lhsT=SEL4[:, 0:40],
                                 rhs=BHNROWS, start=True, stop=False)
                nc.tensor.matmul(out=gh[64:104, 1024:1536],
                                 lhsT=b16[64:104, off["i104"] + 64:
                                          off["i104"] + 104],
                                 rhs=ring[64:104, rb + 1024:rb + 1536],
                                 start=True, stop=False)
            # slice blocks in chain order r, z, n; k-major quads inside each
            # block so the four cells' streams run in four col strips.
            for n_lo in (0, 1024, 512):
                for k in range(4):
                    for cname, base, _ in active:
                        nc.tensor.matmul(
                            out=gh[base:base + BC, n_lo:n_lo + 512],
                            lhsT=hch(base, k),
                            rhs=U[cname][k][:, n_lo:n_lo + 512],
                            start=False, stop=(k == 3),
                            tile_position=(0, base))

            # ---------- bulk gi matmuls: PE gap fillers between this step's
            # MM block and its transposes. gi1 for the L1 wavefront on
            # t0%CH in [0,6); gi0 for the next x-chunk on t0%CH in [6,12).
            # Evacuations are deferred to the end of the iteration.
            evacs = []
            bc = (t0 - CH) // CH          # chunk fully copied CH iters ago
            ph = (t0 - CH) % CH
            if 0 <= bc < NCHUNK and ph < 6:
                lhs = [bufA[bc][:, k, :] for k in range(4)] + \
                      [bufC[bc][:, k, :] for k in range(4)]
                cell = ("f1", "b1")[ph // 3]
                n = ph % 3
                ps = bulk_group(cell, lhs, n)
                evacs.append((ps, gi1_dram[cell][bc], n))
            c0 = t0 // CH + 1             # gi0 for the next chunk
            ph0 = t0 % CH
            if l0 and c0 < NCHUNK:
                if ph0 == 5:
                    x16_next = load_xchunk(c0)
                elif 6 <= ph0 < 12:
                    j = ph0 - 6
                    cell = ("f0", "b0")[j // 3]
                    n = j % 3
                    ps = bulk_group(cell, [x16_next[:, k, :] for k in range(4)], n)
                    evacs.append((ps, gi0_dram[cell][c0 * 128:(c0 + 1) * 128, :], n))

            # ---------- gates: all four cells per op, full 512-col ops ----
            h_prev = h2_prev[:] if h2_prev is not None else zeroH[0:104, :]
            rz = g_pool.tile([104, 1024], BF16, tag="rz")   # r 0:512, z 512:1024
            nc.scalar.activation(rz[:, 0:512], gh[P104, 0:512], ACT.Sigmoid)
            nc.scalar.activation(rz[:, 512:1024], gh[P104, 512:1024], ACT.Sigmoid)

            zb = g_pool.tile([104, 512], BF16, tag="zb")
            m2 = g_pool.tile([104, 512], BF16, tag="m2")
            u = g_pool.tile([104, 512], BF16, tag="u")
            v = g_pool.tile([104, 512], BF16, tag="v")
            nt = g_pool.tile([104, 512], BF16, tag="nt")
            nb = g_pool.tile([104, 512], BF16, tag="nb")
            h2 = h2_pool.tile([104, 512], BF16, tag="h2", name=f"h2_{i}")

            # z-branch on the (otherwise idle) GPSIMD engine so the DVE
            # queue stays short on the critical path.
            nc.gpsimd.tensor_scalar(out=zb[:], in0=rz[:, 512:1024],
                                    scalar1=-1.0, scalar2=1.0,
                                    op0=OP.mult, op1=OP.add)
            nc.gpsimd.tensor_mul(out=m2[:], in0=rz[:, 512:1024],
                                 in1=h_prev[0:104, :])
            nc.vector.tensor_tensor(out=u[:], in0=rz[:, 0:512],
                                    in1=gh[P104, 1024:1536], op=OP.mult)
            nc.vector.tensor_tensor(out=v[:, 0:256], in0=u[:, 0:256],
                                    in1=ring[0:104, rb + 1024:rb + 1280],
                                    op=OP.add)
            nc.vector.tensor_tensor(out=v[:, 256:512], in0=u[:, 256:512],
                                    in1=ring[0:104, rb + 1280:rb + 1536],
                                    op=OP.add)
            nc.scalar.activation(nt[:], v[:], ACT.Tanh)
            nc.vector.tensor_tensor(out=nb[:], in0=nt[:], in1=zb[:],
                                    op=OP.mult)
            nc.vector.tensor_tensor(out=h2[0:104, 0:256], in0=nb[:, 0:256],
                                    in1=m2[:, 0:256], op=OP.add)
            nc.vector.tensor_tensor(out=h2[0:104, 256:512], in0=nb[:, 256:512],
                                    in1=m2[:, 256:512], op=OP.add)

            # PE gap fillers pinned to the gate chain's phase
            keep_warm(2 if evacs else 6, rz[0:8, 0:512])
            keep_warm(2 if evacs else 4, nt[0:8, 0:512])

            # ---------- transpose h2 -> hidden-major for next step ----
            ptrA = ptrp.tile([128, 2, 104], BF16, tag="ptrA", bufs=1,
                             name=f"ptrA{i}")
            ptrB = ptrp.tile([128, 2, 104], BF16, tag="ptrB", bufs=1,
                             name=f"ptrB{i}")
            for k in range(4):
                nc.tensor.transpose(out=(ptrA if k < 2 else ptrB)[:, k % 2, 0:104],
                                    in_=h2[0:104, k * 128:(k + 1) * 128],
                                    identity=I104)
            hT = hT_pool.tile([128, 4, 104], BF16, tag="hT", name=f"hT{i}")
            nc.vector.tensor_copy(out=hT[:, 0:2, :], in_=ptrA[:])
            nc.vector.tensor_copy(out=hT[:, 2:4, :], in_=ptrB[:])
            hT_prev = hT
            h2_prev = h2

            if l1:
                nc.gpsimd.dma_start(out=out_d[t1], in_=h2[64:104, :])
            if l0:
                sl = (t0 % CH) * BC
                nc.gpsimd.tensor_copy(out=bufA[cc0][:, :, sl:sl + BC],
                                      in_=hT[:, :, 0:BC])
                nc.gpsimd.tensor_copy(out=bufC[cc0][:, :, sl:sl + BC],
                                      in_=hT[:, :, 39:31:-1])

            for ps, dst, n in evacs:
                evac_group(ps, dst, n)


# ---------------------------------------------------------------------------
# host side
# ---------------------------------------------------------------------------

_CACHE = {}


def _groups():
    return [list(range(4 * d, 4 * d + 4)) + [63 - (4 * d + 3), 63 - (4 * d + 2),
            63 - (4 * d + 1), 63 - 4 * d] for d in range(NCORES)]


def _bf16_u16(a):
    a = np.ascontiguousarray(a, np.float32)
    u = a.view(np.uint32)
    return ((u + 0x7FFF + ((u >> 16) & 1)) >> 16).astype(np.uint16)


def _pack_words(u16):
    ev = u16[..., 0::2].astype(np.uint32)
    od = u16[..., 1::2].astype(np.uint32)
    return (ev | (od << 16)).view(np.float32)


def _blob_host(inputs):
    off, totw = _blob_layout()
    blob = np.zeros((128, totw), np.uint16)
    one = _bf16_u16(np.ones(1, np.float32))[0]
    for ci, (cname, base, kx) in enumerate(CELLS):
        wih = np.asarray(inputs[f"wih_{cname}"], np.float32)   # (1536, in)
        whh = np.asarray(inputs[f"whh_{cname}"], np.float32)   # (1536, 512)
        bih = np.asarray(inputs[f"bih_{cname}"], np.float32)
        bhh = np.asarray(inputs[f"bhh_{cname}"], np.float32)
        wt = _bf16_u16(wih.T.reshape(kx, 128, G3))
        ut = _bf16_u16(whh.T.reshape(4, 128, G3))
        for k in range(kx):
            o = off[f"w_{cname}{k}"]
            blob[:, o:o + G3] = wt[k]
        for k in range(4):
            o = off[f"u_{cname}{k}"]
            blob[:, o:o + G3] = ut[k]
        bulkb = np.concatenate([(bih + bhh)[:1024], bih[1024:]])
        blob[ci, off["bias"]:off["bias"] + 1536] = _bf16_u16(bulkb)
        blob[ci, off["bias"] + 1536:off["bias"] + 2048] = _bf16_u16(bhh[1024:])
        blob[ci, off["ohrow"] + ci * 128:off["ohrow"] + (ci + 1) * 128] = \
            _bf16_u16(np.ones(128, np.float32))
    for j in range(104):
        blob[j, off["i104"] + j] = one
    for ci, (cname, base, kx) in enumerate(CELLS):
        for j in range(BC):
            blob[ci, off["sel4"] + base + j] = one
    return _pack_words(blob)


def _in_maps(inputs):
    S = inputs["x"].shape[0]
    x = np.asarray(inputs["x"], np.float32)
    groups = _groups()
    blob = _blob_host(inputs)
    in_maps = []
    for d in range(NCORES):
        xl = x[:, groups[d], :]                       # (S, 8, 512)
        # xTp layout: (128 part, 4 k, S*BC) -> words
        xT = _bf16_u16(xl.transpose(2, 0, 1).reshape(4, 128, S * BC))
        xT = np.ascontiguousarray(xT.transpose(1, 0, 2))   # (128, 4, S*BC)
        in_maps.append({"blob": blob, "xTp": _pack_words(xT)})
    return in_maps


def _assemble(outs, S):
    groups = _groups()
    out = np.zeros((S, BATCH, 2 * HID), np.float32)
    for d in range(NCORES):
        raw = np.asarray(outs[d]["out"], np.float32)  # (S, 40, 512)
        G = groups[d]
        for b in range(BC):
            out[:, G[b], 0:HID] = raw[:, b, :]
            out[:, G[b], HID:] = raw[:, 32 + 7 - b, :]
    return out


class _Runner:
    """Caches the traced+compiled SPMD executable so repeat calls skip the
    (expensive) jax retrace and BIR re-serialization."""

    def __init__(self, S):
        import jax
        from jax.sharding import Mesh, PartitionSpec
        from jax.experimental.shard_map import shard_map
        from concourse import bass2jax
        from concourse.bass2jax import _bass_exec_p, partition_id_tensor

        bass2jax.install_neuronx_cc_hook()
        self.S = S
        nc = build_core_program(S)
        self.nc = nc
        partition_name = nc.partition_id_tensor.name if nc.partition_id_tensor else None
        in_names, out_names, out_avals, zero_outs = [], [], [], []
        for alloc in nc.m.functions[0].allocations:
            if not isinstance(alloc, mybir.MemoryLocationSet):
                continue
            name = alloc.memorylocations[0].name
            if alloc.kind == "ExternalInput":
                if name != partition_name:
                    in_names.append(name)
            elif alloc.kind == "ExternalOutput":
                shape = tuple(alloc.tensor_shape)
                dtype = mybir.dt.np(alloc.dtype)
                out_names.append(name)
                out_avals.append(jax.core.ShapedArray(shape, dtype))
                zero_outs.append(np.zeros(shape, dtype))
        n_params = len(in_names)
        self.in_names = list(in_names)
        self.out_names = out_names
        self.out_shapes = [tuple(a.shape) for a in out_avals]
        self.zero_outs = zero_outs
        all_in = in_names + out_names + ([partition_name] if partition_name else [])

        def _body(*args):
            operands = list(args)
            if partition_name is not None:
                operands.append(partition_id_tensor())
            return tuple(_bass_exec_p.bind(
                *operands,
                out_avals=tuple(out_avals),
                in_names=tuple(all_in),
                out_names=tuple(out_names),
                lowering_input_output_aliases=(),
                sim_require_finite=True,
                sim_require_nnan=True,
                nc=nc,
            ))

        devices = jax.devices()[:NCORES]
        mesh = Mesh(np.asarray(devices), ("core",))
        in_specs = (PartitionSpec("core"),) * (n_params + len(out_names))
        out_specs = (PartitionSpec("core"),) * len(out_names)
        self.fn = jax.jit(
            shard_map(_body, mesh=mesh, in_specs=in_specs,
                      out_specs=out_specs, check_rep=False),
            keep_unused=True)
        self.jax = jax

    def run(self, in_maps):
        concat_in = [
            np.concatenate([np.asarray(m[nm]) for m in in_maps], axis=0)
            for nm in self.in_names]
        concat_zero = [np.zeros((NCORES * z.shape[0], *z.shape[1:]), z.dtype)
                       for z in self.zero_outs]
        outs = self.fn(*concat_in, *concat_zero)
        return [
            {nm: np.asarray(outs[i]).reshape(NCORES, *self.out_shapes[i])[c]
             for i, nm in enumerate(self.out_names)}
            for c in range(NCORES)]

    def run_timed(self, in_maps, iters=5):
        """Stage inputs (and the pre-zeroed output buffers — every output
        element is written, so reuse is safe) on device; time executions."""
        import time
        concat_in = [
            self.jax.device_put(np.concatenate(
                [np.asarray(m[nm]) for m in in_maps], axis=0))
            for nm in self.in_names]
        concat_zero = [
            self.jax.device_put(
                np.zeros((NCORES * z.shape[0], *z.shape[1:]), z.dtype))
            for z in self.zero_outs]
        o = self.fn(*concat_in, *concat_zero)
        self.jax.block_until_ready(o)
        best = float("inf")
        for _ in range(iters):
            t0 = time.perf_counter()
            o = self.fn(*concat_in, *concat_zero)
            self.jax.block_until_ready(o)
            best = min(best, time.perf_counter() - t0)
        return best


def kernel(**inputs):
    S = inputs["x"].shape[0]
    if S not in _CACHE:
        _CACHE[S] = _Runner(S)
    runner = _CACHE[S]
    outs = runner.run(_in_maps(inputs))
    return _assemble(outs, S)


if __name__ == "__main__":
    rng = np.random.default_rng(0)
    S = 32
    inputs = {"x": rng.standard_normal((S, 64, 512), dtype=np.float32)}
    s = 1.0 / np.sqrt(HID)
    u = lambda *shp: rng.uniform(-s, s, shp).astype(np.float32)
    for c, idim in (("f0", 512), ("b0", 512), ("f1", 1024), ("b1", 1024)):
        inputs[f"wih_{c}"] = u(G3, idim)
        inputs[f"whh_{c}"] = u(G3, HID)
        inputs[f"bih_{c}"] = u(G3)
        inputs[f"bhh_{c}"] = u(G3)
    out = kernel(**inputs)
    print("kernel ran, out", out.shape, float(np.abs(out).mean()))
